# revision 1
# baseline (speedup 1.0000x reference)
"""Trainium2 Bass kernel for MoGNN forward (global mean-pool + linear).

The model's conv outputs are discarded; the result depends only on x:
    pooled[g] = mean over nodes n with batch[n] == g of x[n]   # [1024, 512]
    out = pooled @ W.T + b                                     # [1024, 7]

batch ids are sorted, so nodes of each graph are contiguous. We shard by
GRAPHS: core k owns graphs [128k, 128k+128) and exactly the contiguous row
range of x belonging to them (padded to a tile multiple). No collectives.

Per 128-node tile, on device:
  - DVE builds an exact one-hot matrix oh[n, g] = (batch_local[n] == g);
    one tensor_tensor(is_equal) per DMA chunk via step-0 broadcast APs.
  - PE matmul (fp16 in, fp32 PSUM accumulate, full rate at N=512) does
    psum[128 graphs, 512 feats] += oh.T @ x_tile.
Epilogue (all fp16 for full-rate PE): PSUM -> SBUF with a per-graph 1/count
scale (mean pool), 4x PE transpose to feat-major (pipelined behind the
sliced scale), then 4 fp16 matmuls with pooled.T stationary and the W chunk
moving (N=7, fp32 PSUM), bias added via a partition-replicated fp32 tile;
each core writes out[128, 7] and the host concatenates to [1024, 7].

x is shipped as fp16 (11-bit effective mantissa; accumulation stays fp32 in
PSUM) — measured end-to-end relative error vs the fp32 reference ~2e-4,
comparable to the fp32r (tf32-like) matmul path while halving HBM traffic.
"""

import numpy as np

NCORES = 8
G = 1024            # total graphs
GPC = G // NCORES   # graphs per core = 128
F = 512             # feature dim
P = 128             # partition / node-tile size
CHUNK = 8           # node tiles per DMA chunk (1 MB fp16 transfers)

_compiled_cache = {}


def _chunk_plan(ntiles):
    """Chunk boundaries: small leading chunks so the PE pipeline starts early,
    CHUNK-tile steady state, and a small taper at the end so the PE finishes
    right behind the final DMA bytes."""
    head = [min(2, CHUNK), min(6, CHUNK)]
    tail = [min(2, CHUNK)]
    main_end = max(ntiles - sum(tail), 0)
    chunks = []
    t0 = 0
    for ramp in head:
        if t0 < main_end:
            clen = min(ramp, main_end - t0)
            chunks.append((t0, clen))
            t0 += clen
    while t0 < main_end:
        clen = min(CHUNK, main_end - t0)
        chunks.append((t0, clen))
        t0 += clen
    for ramp in tail:
        if t0 < ntiles:
            clen = min(ramp, ntiles - t0)
            chunks.append((t0, clen))
            t0 += clen
    while t0 < ntiles:
        clen = min(CHUNK, ntiles - t0)
        chunks.append((t0, clen))
        t0 += clen
    assert sum(c for _, c in chunks) == ntiles
    return chunks


def _build(ntiles):
    """Build + compile the per-core Bass kernel for a shard of `ntiles` node tiles."""
    from concourse import bacc, tile, mybir

    f32 = mybir.dt.float32
    f16 = mybir.dt.float16
    eq = mybir.AluOpType.is_equal
    mult = mybir.AluOpType.mult
    add = mybir.AluOpType.add

    nrows = ntiles * P
    chunks = _chunk_plan(ntiles)

    nc = bacc.Bacc(
        "TRN2",
        target_bir_lowering=False,
        debug=False,
        num_devices=NCORES,
    )

    # x shard laid out chunk-contiguous and partition-major inside each chunk:
    # for chunk (c0, clen), the DRAM block holds block[p, t, m] = x[(c0+t)*128+p, m]
    # so the whole chunk is one contiguous region and each partition reads one
    # contiguous multi-KB run
    x_d = nc.dram_tensor("xs", [nrows * F], f16, kind="ExternalInput")
    # constants packed into two tensors (one DMA each, on the scalar-engine
    # HWDGE ring so they don't block the x-chunk FIFO on the sync ring):
    # cp16 = [bl | iota], cp32 = [ident | wtr | b | icnt]
    # cp16 = [bl | iota | ident | wtr], cp32 = [b_replicated | icnt]
    cp16_d = nc.dram_tensor(
        "cp16", [P, ntiles + GPC + P + 28], f16, kind="ExternalInput"
    )
    cp32_d = nc.dram_tensor("cp32", [P, 8], f32, kind="ExternalInput")
    out_d = nc.dram_tensor("out", [GPC, 7], f32, kind="ExternalOutput")

    with tile.TileContext(nc) as tc:
        with (
            tc.tile_pool(name="const", bufs=1) as constp,
            tc.tile_pool(name="xin", bufs=4) as xp,
            tc.tile_pool(name="oh", bufs=4) as ohp,
            tc.tile_pool(name="acc", bufs=1, space="PSUM") as accp,
            tc.tile_pool(name="tps", bufs=2, space="PSUM") as tpsp,
            tc.tile_pool(name="outp", bufs=1, space="PSUM") as outpp,
            tc.tile_pool(name="sb", bufs=2) as sbp,
        ):
            cp16_t = constp.tile([P, ntiles + GPC + P + 28], f16)
            nc.scalar.dma_start(cp16_t[:], cp16_d.ap())
            cp32_t = constp.tile([P, 8], f32)
            nc.scalar.dma_start(cp32_t[:], cp32_d.ap())
            bl_t = cp16_t[:, 0:ntiles]
            iota_t = cp16_t[:, ntiles : ntiles + GPC]
            ident_t = cp16_t[:, ntiles + GPC : ntiles + GPC + P]
            wtr_t = cp16_t[:, ntiles + GPC + P : ntiles + GPC + P + 28]
            brep_t = cp32_t[:, 0:7]
            icnt_t = cp32_t[:, 7:8]

            acc = accp.tile([GPC, F], f32)
            x_flat = x_d.ap()

            iota_rep = iota_t.rearrange("p (a g) -> p a g", a=1)
            t = 0
            for c0, clen in chunks:
                xt = xp.tile([P, CHUNK, F], f16, tag="xt")
                chunk_ap = x_flat[c0 * P * F : (c0 + clen) * P * F].rearrange(
                    "(p t m) -> p t m", p=P, m=F
                )
                nc.sync.dma_start(xt[:, :clen, :], chunk_ap)
                # one-hot for the whole chunk in one DVE op via broadcast APs:
                # oh[p, n, g] = (iota[g] == bl[p, c0+n])
                oh = ohp.tile([P, CHUNK, GPC], f16, tag="oh")
                nc.vector.tensor_tensor(
                    oh[:, :clen, :],
                    iota_rep.broadcast_to([P, clen, GPC]),
                    bl_t[:, c0 : c0 + clen]
                    .rearrange("p (n a) -> p n a", a=1)
                    .broadcast_to([P, clen, GPC]),
                    op=eq,
                )
                for n in range(clen):
                    nc.tensor.matmul(
                        acc[:],
                        oh[:, n, :],
                        xt[:, n, :],
                        start=(t == 0),
                        stop=(t == ntiles - 1),
                    )
                    t += 1

            # pooled = acc * (1/count[g]) cast to fp16, sliced so the (fp16,
            # full-rate) transposes pipeline behind the scale copies; then the
            # classifier with pooled.T as stationary (moving is W [128, 7], N=7)
            pooled = sbp.tile([GPC, F], f16)
            ptall = sbp.tile([P, 4, P], f16)
            for j in range(4):
                sl = slice(j * P, (j + 1) * P)
                nc.vector.tensor_scalar(
                    pooled[:, sl], acc[:, sl], icnt_t, None, op0=mult
                )
                tp = tpsp.tile([P, P], f16)
                nc.tensor.transpose(tp[:], pooled[:, sl], ident_t)
                nc.vector.tensor_copy(ptall[:, j, :], tp[:])

            out_ps = outpp.tile([GPC, 7], f32)
            for j in range(4):
                nc.tensor.matmul(
                    out_ps[:],
                    ptall[:, j, :],
                    wtr_t[:, j * 7 : (j + 1) * 7],
                    start=(j == 0),
                    stop=(j == 3),
                )

            out_sb = sbp.tile([GPC, 7], f32)
            nc.vector.tensor_tensor(out_sb[:], out_ps[:], brep_t, op=add)
            nc.sync.dma_start(out_d.ap(), out_sb[:])

    nc.compile()
    return nc


def _get_compiled(ntiles):
    if ntiles not in _compiled_cache:
        _compiled_cache[ntiles] = _build(ntiles)
    return _compiled_cache[ntiles]


def _prep_in_maps(x16, batch, W, b, ntiles, bounds, inv_counts):
    cap = ntiles * P
    chunk_plan = _chunk_plan(ntiles)
    iota = np.tile(np.arange(GPC, dtype=np.float16)[None, :], (P, 1))
    # wtr[p, c*7+j] = W.T[c*128+p, j]
    wtr = np.ascontiguousarray(
        W.T.reshape(4, P, 7).transpose(1, 0, 2).reshape(P, 28)
    ).astype(np.float16)
    cp32_base = np.zeros((P, 8), dtype=np.float32)
    cp32_base[:, 0:7] = b.astype(np.float32)[None, :]

    in_maps = []
    for k in range(NCORES):
        lo, hi = int(bounds[k]), int(bounds[k + 1])
        n = hi - lo
        xs = np.zeros((cap, F), dtype=np.float16)
        xs[:n] = x16[lo:hi]
        # chunk-contiguous, partition-major within each chunk
        xs = xs.reshape(ntiles, P, F)
        parts = [
            np.ascontiguousarray(xs[c0 : c0 + clen].transpose(1, 0, 2)).reshape(-1)
            for c0, clen in chunk_plan
        ]
        xs = np.concatenate(parts)
        blv = np.full((cap,), -1.0, dtype=np.float16)
        blv[:n] = (batch[lo:hi] - GPC * k).astype(np.float16)
        cp16 = np.empty((P, ntiles + GPC + P + 28), dtype=np.float16)
        cp16[:, 0:ntiles] = blv.reshape(ntiles, P).T
        cp16[:, ntiles : ntiles + GPC] = iota
        cp16[:, ntiles + GPC : ntiles + GPC + P] = np.eye(P, dtype=np.float16)
        cp16[:, ntiles + GPC + P :] = wtr
        cp32 = cp32_base.copy()
        cp32[:, 7] = inv_counts[GPC * k : GPC * (k + 1)]
        in_maps.append({"xs": xs, "cp16": cp16, "cp32": cp32})
    return in_maps


_last_result = None  # test harness can read exec_time_ns / trace from here


def kernel(x, edge_index, edge_attr, batch_size, W, b):
    from concourse import bass_utils

    global _last_result

    x16 = np.asarray(x, dtype=np.float16)
    batch = np.asarray(batch_size).astype(np.int64)
    W = np.asarray(W, dtype=np.float32)
    b = np.asarray(b, dtype=np.float32)

    if batch.size > 1 and np.any(np.diff(batch) < 0):
        # contiguous-shard logic needs sorted ids; reordering nodes does not
        # change per-graph sums
        order = np.argsort(batch, kind="stable")
        batch = batch[order]
        x16 = x16[order]

    counts = np.bincount(batch, minlength=G)
    inv_counts = (1.0 / np.maximum(counts, 1)).astype(np.float32)
    bounds = np.searchsorted(batch, np.arange(0, G + 1, GPC))
    max_rows = int(np.diff(bounds).max())
    ntiles = max(-(-max_rows // P), 1)

    nc = _get_compiled(ntiles)
    in_maps = _prep_in_maps(x16, batch, W, b, ntiles, bounds, inv_counts)

    res = bass_utils.run_bass_kernel_spmd(
        nc, in_maps, core_ids=list(range(NCORES))
    )
    _last_result = res

    # each core returns out [128, 7] for its graphs; assemble [1024, 7]
    out = np.concatenate(
        [np.asarray(res.results[k]["out"]) for k in range(NCORES)], axis=0
    )
    return np.ascontiguousarray(out.astype(np.float32))



# revision 3
# speedup vs baseline: 1.0942x; 1.0942x over previous
"""Trainium2 Bass kernel for MoGNN forward (global mean-pool + linear).

The model's conv outputs are discarded; the result depends only on x:
    pooled[g] = mean over nodes n with batch[n] == g of x[n]   # [1024, 512]
    out = pooled @ W.T + b                                     # [1024, 7]

batch ids are sorted, so nodes of each graph are contiguous. We shard by
GRAPHS: core k owns graphs [128k, 128k+128) and exactly the contiguous row
range of x belonging to them (padded to a tile multiple). No collectives.

Per 128-node tile, on device:
  - DVE builds an exact one-hot matrix oh[n, g] = (batch_local[n] == g);
    one tensor_tensor(is_equal) per DMA chunk via step-0 broadcast APs.
  - PE matmul (fp16 in, fp32 PSUM accumulate, full rate at N=512) does
    psum[128 graphs, 512 feats] += oh.T @ x_tile.

The x stream saturates HBM (~358 GB/s/core); everything else hides under
it. PE start latency is minimized by shipping only the 2 data-dependent
small constants (per-tile batch ids `bl` + W.T chunks, and a [128,2] f32
with 1/count + bias) on the scalar DMA ring, while the iota row and the
transpose identity are generated on device (gpsimd iota + one DVE
is_equal) so the first one-hot only waits on the ~34KB blw DMA.

Epilogue (after the last accumulate): per 128-col feature block j, a
scale+cast (acc * 1/count -> fp16), alternating scalar/vector engines so
two blocks proceed in parallel; PE transpose to feature-major; PSUM->SBUF
copy on the engine opposite the block's scale; then 4 fp16 matmuls with
the W.T chunk stationary and pooled.T moving, accumulating out.T [7, 128]
in PSUM. Bias is added on the scalar engine (per-partition bias AP) and
the [7, 128] f32 result is written with 7 fat DMA packets (vs 128x28B).
Host transposes and concatenates the 8 core outputs.

x is shipped as fp16 (11-bit effective mantissa; accumulation stays fp32
in PSUM) - measured end-to-end relative error vs the fp32 reference
~3e-4, while halving HBM traffic vs fp32.
"""

import numpy as np

NCORES = 8
G = 1024            # total graphs
GPC = G // NCORES   # graphs per core = 128
F = 512             # feature dim
P = 128             # partition / node-tile size
CHUNK = 8           # node tiles per DMA chunk (1 MB fp16 transfers)

_compiled_cache = {}


def _chunk_plan(ntiles):
    """Chunk boundaries: small leading chunks so the PE pipeline starts early,
    CHUNK-tile steady state, and a small taper at the end so the PE finishes
    right behind the final DMA bytes."""
    head = [min(2, CHUNK), min(6, CHUNK)]
    tail = [min(2, CHUNK)]
    main_end = max(ntiles - sum(tail), 0)
    chunks = []
    t0 = 0
    for ramp in head:
        if t0 < main_end:
            clen = min(ramp, main_end - t0)
            chunks.append((t0, clen))
            t0 += clen
    while t0 < main_end:
        clen = min(CHUNK, main_end - t0)
        chunks.append((t0, clen))
        t0 += clen
    for ramp in tail:
        if t0 < ntiles:
            clen = min(ramp, ntiles - t0)
            chunks.append((t0, clen))
            t0 += clen
    while t0 < ntiles:
        clen = min(CHUNK, ntiles - t0)
        chunks.append((t0, clen))
        t0 += clen
    assert sum(c for _, c in chunks) == ntiles
    return chunks


def _build(ntiles):
    """Build + compile the per-core Bass kernel for a shard of `ntiles` node tiles."""
    from concourse import bacc, tile, mybir

    f32 = mybir.dt.float32
    f16 = mybir.dt.float16
    eq = mybir.AluOpType.is_equal
    mult = mybir.AluOpType.mult

    nrows = ntiles * P
    chunks = _chunk_plan(ntiles)

    nc = bacc.Bacc(
        "TRN2",
        target_bir_lowering=False,
        debug=False,
        num_devices=NCORES,
    )

    # x shard laid out chunk-contiguous and partition-major inside each chunk:
    # for chunk (c0, clen), the DRAM block holds block[p, t, m] = x[(c0+t)*128+p, m]
    # so the whole chunk is one contiguous region and each partition reads one
    # contiguous multi-KB run
    x_d = nc.dram_tensor("xs", [nrows * F], f16, kind="ExternalInput")
    # small data-dependent constants on the scalar-ring queues (don't block
    # the x-chunk FIFO): blw = [bl | wtr], cp32 = [icnt | bcol]
    blw_d = nc.dram_tensor("blw", [P, ntiles + 28], f16, kind="ExternalInput")
    cp32_d = nc.dram_tensor("cp32", [P, 2], f32, kind="ExternalInput")
    out_d = nc.dram_tensor("out", [7, GPC], f32, kind="ExternalOutput")

    with tile.TileContext(nc) as tc:
        with (
            tc.tile_pool(name="const", bufs=1) as constp,
            tc.tile_pool(name="xin", bufs=4) as xp,
            tc.tile_pool(name="oh", bufs=4) as ohp,
            tc.tile_pool(name="acc", bufs=1, space="PSUM") as accp,
            tc.tile_pool(name="tps", bufs=2, space="PSUM") as tpsp,
            tc.tile_pool(name="outp", bufs=1, space="PSUM") as outpp,
            tc.tile_pool(name="sb", bufs=1) as sbp,
        ):
            blw_t = constp.tile([P, ntiles + 28], f16, tag="blw")
            nc.scalar.dma_start(blw_t[:], blw_d.ap())
            cp32_t = constp.tile([P, 2], f32, tag="cp32")
            nc.scalar.dma_start(cp32_t[:], cp32_d.ap())
            bl_t = blw_t[:, 0:ntiles]
            wtr_t = blw_t[:, ntiles : ntiles + 28]
            icnt_t = cp32_t[:, 0:1]
            bcol_t = cp32_t[0:7, 1:2]

            # on-device constants: iota row (one-hot compare) + transpose identity
            iota_t = constp.tile([P, GPC], f16, tag="iota")
            nc.gpsimd.iota(
                iota_t[:], [[1, GPC]], base=0, channel_multiplier=0,
                allow_small_or_imprecise_dtypes=True,
            )
            pidx_t = constp.tile([P, 1], f32, tag="pidx")
            nc.gpsimd.iota(
                pidx_t[:], [[0, 1]], base=0, channel_multiplier=1,
                allow_small_or_imprecise_dtypes=True,
            )
            ident_t = constp.tile([P, P], f16, tag="ident")
            nc.vector.tensor_scalar(ident_t[:], iota_t[:, 0:P], pidx_t, None, op0=eq)

            acc = accp.tile([GPC, F], f32)
            x_flat = x_d.ap()

            iota_rep = iota_t[:].rearrange("p (a g) -> p a g", a=1)
            t = 0
            for c0, clen in chunks:
                xt = xp.tile([P, CHUNK, F], f16, tag="xt")
                chunk_ap = x_flat[c0 * P * F : (c0 + clen) * P * F].rearrange(
                    "(p t m) -> p t m", p=P, m=F
                )
                nc.sync.dma_start(xt[:, :clen, :], chunk_ap)
                # one-hot for the whole chunk in one DVE op via broadcast APs:
                # oh[p, n, g] = (iota[g] == bl[p, c0+n])
                oh = ohp.tile([P, CHUNK, GPC], f16, tag="oh")
                nc.vector.tensor_tensor(
                    oh[:, :clen, :],
                    iota_rep.broadcast_to([P, clen, GPC]),
                    bl_t[:, c0 : c0 + clen]
                    .rearrange("p (n a) -> p n a", a=1)
                    .broadcast_to([P, clen, GPC]),
                    op=eq,
                )
                for n in range(clen):
                    nc.tensor.matmul(
                        acc[:],
                        oh[:, n, :],
                        xt[:, n, :],
                        start=(t == 0),
                        stop=(t == ntiles - 1),
                    )
                    t += 1

            # pooled = acc * (1/count[g]) cast to fp16; scale+cast alternates
            # scalar/vector engines, the PSUM->SBUF copy runs on the opposite
            # engine, and the (fp16, full-rate) PE transposes pipeline between
            pooled = sbp.tile([GPC, F], f16, tag="pooled")
            ptall = sbp.tile([P, 4, P], f16, tag="ptall")
            for j in range(4):
                sl = slice(j * P, (j + 1) * P)
                if j % 2 == 0:
                    nc.scalar.mul(pooled[:, sl], acc[:, sl], icnt_t)
                else:
                    nc.vector.tensor_scalar(
                        pooled[:, sl], acc[:, sl], icnt_t, None, op0=mult
                    )
                tp = tpsp.tile([P, P], f16, tag="tp")
                nc.tensor.transpose(tp[:], pooled[:, sl], ident_t)
                if j % 2 == 0:
                    nc.vector.tensor_copy(ptall[:, j, :], tp[:])
                else:
                    nc.scalar.copy(ptall[:, j, :], tp[:])

            # classifier: out.T[j, g] = sum_m W.T[m, j] * pooled.T[m, g],
            # W.T chunk stationary [128, 7], pooled.T chunk moving [128, 128]
            out_ps = outpp.tile([7, GPC], f32)
            for j in range(4):
                nc.tensor.matmul(
                    out_ps[:],
                    wtr_t[:, j * 7 : (j + 1) * 7],
                    ptall[:, j, :],
                    start=(j == 0),
                    stop=(j == 3),
                )

            out_sb = sbp.tile([7, GPC], f32, tag="outsb")
            nc.scalar.add(out_sb[:], out_ps[:], bcol_t)
            nc.sync.dma_start(out_d.ap(), out_sb[:])

    nc.compile()
    return nc


def _get_compiled(ntiles):
    if ntiles not in _compiled_cache:
        _compiled_cache[ntiles] = _build(ntiles)
    return _compiled_cache[ntiles]


def _prep_in_maps(x16, batch, W, b, ntiles, bounds, inv_counts):
    cap = ntiles * P
    chunk_plan = _chunk_plan(ntiles)
    # wtr[p, c*7+j] = W.T[c*128+p, j]
    wtr = np.ascontiguousarray(
        W.T.reshape(4, P, 7).transpose(1, 0, 2).reshape(P, 28)
    ).astype(np.float16)

    in_maps = []
    for k in range(NCORES):
        lo, hi = int(bounds[k]), int(bounds[k + 1])
        n = hi - lo
        xs = np.zeros((cap, F), dtype=np.float16)
        xs[:n] = x16[lo:hi]
        # chunk-contiguous, partition-major within each chunk
        xs = xs.reshape(ntiles, P, F)
        parts = [
            np.ascontiguousarray(xs[c0 : c0 + clen].transpose(1, 0, 2)).reshape(-1)
            for c0, clen in chunk_plan
        ]
        xs = np.concatenate(parts)
        blv = np.full((cap,), -1.0, dtype=np.float16)
        blv[:n] = (batch[lo:hi] - GPC * k).astype(np.float16)
        blw = np.empty((P, ntiles + 28), dtype=np.float16)
        blw[:, 0:ntiles] = blv.reshape(ntiles, P).T
        blw[:, ntiles:] = wtr
        cp32 = np.zeros((P, 2), dtype=np.float32)
        cp32[:, 0] = inv_counts[GPC * k : GPC * (k + 1)]
        cp32[0:7, 1] = b.astype(np.float32)
        in_maps.append({"xs": xs, "blw": blw, "cp32": cp32})
    return in_maps


_last_result = None  # test harness can read exec_time_ns / trace from here


def kernel(x, edge_index, edge_attr, batch_size, W, b):
    from concourse import bass_utils

    global _last_result

    x16 = np.asarray(x, dtype=np.float16)
    batch = np.asarray(batch_size).astype(np.int64)
    W = np.asarray(W, dtype=np.float32)
    b = np.asarray(b, dtype=np.float32)

    if batch.size > 1 and np.any(np.diff(batch) < 0):
        # contiguous-shard logic needs sorted ids; reordering nodes does not
        # change per-graph sums
        order = np.argsort(batch, kind="stable")
        batch = batch[order]
        x16 = x16[order]

    counts = np.bincount(batch, minlength=G)
    inv_counts = (1.0 / np.maximum(counts, 1)).astype(np.float32)
    bounds = np.searchsorted(batch, np.arange(0, G + 1, GPC))
    max_rows = int(np.diff(bounds).max())
    ntiles = max(-(-max_rows // P), 1)

    nc = _get_compiled(ntiles)
    in_maps = _prep_in_maps(x16, batch, W, b, ntiles, bounds, inv_counts)

    res = bass_utils.run_bass_kernel_spmd(
        nc, in_maps, core_ids=list(range(NCORES))
    )
    _last_result = res

    # each core returns out.T [7, 128] for its graphs; assemble [1024, 7]
    out = np.concatenate(
        [np.asarray(res.results[k]["out"]).T for k in range(NCORES)], axis=0
    )
    return np.ascontiguousarray(out.astype(np.float32))


# revision 5
# speedup vs baseline: 1.1131x; 1.0173x over previous
"""Trainium2 Bass kernel for MoGNN forward (global mean-pool + linear).

The model's conv outputs are discarded; the result depends only on x:
    pooled[g] = mean over nodes n with batch[n] == g of x[n]   # [1024, 512]
    out = pooled @ W.T + b                                     # [1024, 7]

batch ids are sorted, so nodes of each graph are contiguous. We shard by
GRAPHS: core k owns graphs [128k, 128k+128) and exactly the contiguous row
range of x belonging to them (padded to a tile multiple). No collectives.

Per 128-node tile, on device:
  - DVE builds an exact one-hot matrix oh[n, g] = (batch_local[n] == g);
    one tensor_tensor(is_equal) per DMA chunk via step-0 broadcast APs.
  - PE matmul (fp16 in, fp32 PSUM accumulate, full rate at N=512) does
    psum[128 graphs, 512 feats] += oh.T @ x_tile.

The x stream saturates HBM (~360+ GB/s/core); everything else must hide
under it. All data-dependent constants (per-tile batch ids bl, W.T chunks,
and [1/count | bias] fp32 bitcast to f16 pairs) ride as a 264B per-partition
HEADER inside chunk 0's contiguous packets — zero extra DMA packets, so the
PE's first matmul starts as soon as chunk 0 lands (~1.5us after the first
trigger) instead of waiting ~3us for small constant packets to round-robin
against the saturated x queues. The iota row (one-hot compare) and the
transpose identity are generated on device (gpsimd iota + one DVE is_equal).

Epilogue (after the last accumulate): per 128-col feature block j, DVE does
scale+cast (acc * 1/count -> fp16), PE transposes to feature-major, the
scalar engine copies PSUM->SBUF, then 4 fp16 matmuls with the W.T chunk
stationary and pooled.T moving accumulate out.T [7, 128] in PSUM; bias is
added on DVE and the [7, 128] f32 result goes out in 7 fat DMA packets.
Host transposes and concatenates the 8 core outputs.

x is shipped as fp16 (11-bit effective mantissa; accumulation stays fp32 in
PSUM) - measured end-to-end relative error vs the fp32 reference ~3e-4,
while halving HBM traffic vs fp32.
"""

import numpy as np

NCORES = 8
G = 1024            # total graphs
GPC = G // NCORES   # graphs per core = 128
F = 512             # feature dim
P = 128             # partition / node-tile size
CHUNK = 8           # node tiles per DMA chunk (1 MB fp16 transfers)

_compiled_cache = {}


def _hdr_cols(ntiles):
    # per-partition header in chunk 0: bl [ntiles] | wtr [28] | cp32 [4 f16]
    # bl padded to even so the f32 bitcast view of cp32 stays 4B-aligned
    blc = ntiles + (ntiles & 1)
    return blc, blc + 32


def _chunk_plan(ntiles):
    """Chunk boundaries: small leading chunks so the PE pipeline starts early,
    CHUNK-tile steady state, and a small taper at the end so the PE finishes
    right behind the final DMA bytes."""
    head = [min(2, CHUNK), min(6, CHUNK)]
    tail = [min(2, CHUNK)]
    main_end = max(ntiles - sum(tail), 0)
    chunks = []
    t0 = 0
    for ramp in head:
        if t0 < main_end:
            clen = min(ramp, main_end - t0)
            chunks.append((t0, clen))
            t0 += clen
    while t0 < main_end:
        clen = min(CHUNK, main_end - t0)
        chunks.append((t0, clen))
        t0 += clen
    for ramp in tail:
        if t0 < ntiles:
            clen = min(ramp, ntiles - t0)
            chunks.append((t0, clen))
            t0 += clen
    while t0 < ntiles:
        clen = min(CHUNK, ntiles - t0)
        chunks.append((t0, clen))
        t0 += clen
    assert sum(c for _, c in chunks) == ntiles
    return chunks


def _build(ntiles):
    """Build + compile the per-core Bass kernel for a shard of `ntiles` node tiles."""
    from concourse import bacc, tile, mybir

    f32 = mybir.dt.float32
    f16 = mybir.dt.float16
    eq = mybir.AluOpType.is_equal
    mult = mybir.AluOpType.mult
    add = mybir.AluOpType.add

    nrows = ntiles * P
    chunks = _chunk_plan(ntiles)
    blc, hdr = _hdr_cols(ntiles)

    nc = bacc.Bacc(
        "TRN2",
        target_bir_lowering=False,
        debug=False,
        num_devices=NCORES,
    )

    # x shard laid out chunk-contiguous and partition-major inside each chunk:
    # for chunk (c0, clen), the DRAM block holds block[p, t, m] = x[(c0+t)*128+p, m]
    # so the whole chunk is one contiguous region and each partition reads one
    # contiguous multi-KB run. Chunk 0 additionally carries an hdr-column
    # constant header per partition (bl | wtr | [icnt|b] bitcast as f16 x4).
    x_d = nc.dram_tensor("xs", [nrows * F + P * hdr], f16, kind="ExternalInput")
    out_d = nc.dram_tensor("out", [7, GPC], f32, kind="ExternalOutput")

    with tile.TileContext(nc) as tc:
        with (
            tc.tile_pool(name="const", bufs=1) as constp,
            tc.tile_pool(name="xin", bufs=4) as xp,
            tc.tile_pool(name="oh", bufs=4) as ohp,
            tc.tile_pool(name="acc", bufs=1, space="PSUM") as accp,
            tc.tile_pool(name="tps", bufs=2, space="PSUM") as tpsp,
            tc.tile_pool(name="outp", bufs=1, space="PSUM") as outpp,
            tc.tile_pool(name="sb", bufs=1) as sbp,
        ):
            # on-device constants: iota row (one-hot compare) + transpose identity
            iota_t = constp.tile([P, GPC], f16, tag="iota")
            nc.gpsimd.iota(
                iota_t[:], [[1, GPC]], base=0, channel_multiplier=0,
                allow_small_or_imprecise_dtypes=True,
            )
            pidx_t = constp.tile([P, 1], f32, tag="pidx")
            nc.gpsimd.iota(
                pidx_t[:], [[0, 1]], base=0, channel_multiplier=1,
                allow_small_or_imprecise_dtypes=True,
            )
            ident_t = constp.tile([P, P], f16, tag="ident")
            nc.vector.tensor_scalar(ident_t[:], iota_t[:, 0:P], pidx_t, None, op0=eq)

            acc = accp.tile([GPC, F], f32)
            x_flat = x_d.ap()

            iota_rep = iota_t[:].rearrange("p (a g) -> p a g", a=1)
            t = 0
            xt0 = None
            off = 0
            for ci, (c0, clen) in enumerate(chunks):
                if ci == 0:
                    # chunk 0: [P, hdr + clen*F] with the constant header
                    xt0 = xp.tile([P, hdr + CHUNK * F], f16, tag="xt0")
                    sz = P * (hdr + clen * F)
                    chunk_ap = x_flat[off : off + sz].rearrange(
                        "(p m) -> p m", p=P
                    )
                    nc.sync.dma_start(xt0[:, : hdr + clen * F], chunk_ap)
                    off += sz
                    xt_of = lambda n: xt0[:, hdr + n * F : hdr + (n + 1) * F]
                else:
                    xt = xp.tile([P, CHUNK, F], f16, tag="xt")
                    sz = P * clen * F
                    chunk_ap = x_flat[off : off + sz].rearrange(
                        "(p t m) -> p t m", p=P, m=F
                    )
                    nc.sync.dma_start(xt[:, :clen, :], chunk_ap)
                    off += sz
                    xt_of = lambda n, _xt=xt: _xt[:, n, :]

                bl_t = xt0[:, 0:blc]
                # one-hot for the whole chunk in one DVE op via broadcast APs:
                # oh[p, n, g] = (iota[g] == bl[p, c0+n])
                oh = ohp.tile([P, CHUNK, GPC], f16, tag="oh")
                nc.vector.tensor_tensor(
                    oh[:, :clen, :],
                    iota_rep.broadcast_to([P, clen, GPC]),
                    bl_t[:, c0 : c0 + clen]
                    .rearrange("p (n a) -> p n a", a=1)
                    .broadcast_to([P, clen, GPC]),
                    op=eq,
                )
                for n in range(clen):
                    nc.tensor.matmul(
                        acc[:],
                        oh[:, n, :],
                        xt_of(n),
                        start=(t == 0),
                        stop=(t == ntiles - 1),
                    )
                    t += 1

            wtr_t = xt0[:, blc : blc + 28]
            cpv = xt0[:, blc + 28 : blc + 32].bitcast(f32)  # [P, 2] f32
            icnt_t = cpv[:, 0:1]
            bcol_t = cpv[0:7, 1:2]

            # pooled = acc * (1/count[g]) cast to fp16 on DVE; PE transposes to
            # feature-major; scalar engine copies PSUM->SBUF behind each
            pooled = sbp.tile([GPC, F], f16, tag="pooled")
            ptall = sbp.tile([P, 4, P], f16, tag="ptall")
            for j in range(4):
                sl = slice(j * P, (j + 1) * P)
                nc.vector.tensor_scalar(
                    pooled[:, sl], acc[:, sl], icnt_t, None, op0=mult
                )
                tp = tpsp.tile([P, P], f16, tag="tp")
                nc.tensor.transpose(tp[:], pooled[:, sl], ident_t)
                nc.scalar.copy(ptall[:, j, :], tp[:])

            # classifier: out.T[j, g] = sum_m W.T[m, j] * pooled.T[m, g],
            # W.T chunk stationary [128, 7], pooled.T chunk moving [128, 128]
            out_ps = outpp.tile([7, GPC], f32)
            for j in range(4):
                nc.tensor.matmul(
                    out_ps[:],
                    wtr_t[:, j * 7 : (j + 1) * 7],
                    ptall[:, j, :],
                    start=(j == 0),
                    stop=(j == 3),
                )

            out_sb = sbp.tile([7, GPC], f32, tag="outsb")
            nc.vector.tensor_scalar(out_sb[:], out_ps[:], bcol_t, None, op0=add)
            nc.sync.dma_start(out_d.ap(), out_sb[:])

    nc.compile()
    return nc


def _get_compiled(ntiles):
    if ntiles not in _compiled_cache:
        _compiled_cache[ntiles] = _build(ntiles)
    return _compiled_cache[ntiles]


def _prep_in_maps(x16, batch, W, b, ntiles, bounds, inv_counts):
    cap = ntiles * P
    chunk_plan = _chunk_plan(ntiles)
    blc, hdr = _hdr_cols(ntiles)
    # wtr[p, c*7+j] = W.T[c*128+p, j]
    wtr = np.ascontiguousarray(
        W.T.reshape(4, P, 7).transpose(1, 0, 2).reshape(P, 28)
    ).astype(np.float16)

    in_maps = []
    for k in range(NCORES):
        lo, hi = int(bounds[k]), int(bounds[k + 1])
        n = hi - lo
        xs = np.zeros((cap, F), dtype=np.float16)
        xs[:n] = x16[lo:hi]
        xs = xs.reshape(ntiles, P, F)

        blv = np.full((cap,), -1.0, dtype=np.float16)
        blv[:n] = (batch[lo:hi] - GPC * k).astype(np.float16)
        cp32 = np.zeros((P, 2), dtype=np.float32)
        cp32[:, 0] = inv_counts[GPC * k : GPC * (k + 1)]
        cp32[0:7, 1] = b.astype(np.float32)
        head = np.zeros((P, hdr), dtype=np.float16)
        head[:, 0:ntiles] = blv.reshape(ntiles, P).T
        head[:, blc : blc + 28] = wtr
        head[:, blc + 28 : blc + 32] = cp32.view(np.float16)

        # chunk-contiguous, partition-major within each chunk; chunk 0 gets
        # the constant header prepended per partition
        parts = []
        for ci, (c0, clen) in enumerate(chunk_plan):
            blk = np.ascontiguousarray(
                xs[c0 : c0 + clen].transpose(1, 0, 2)
            ).reshape(P, clen * F)
            if ci == 0:
                blk = np.concatenate([head, blk], axis=1)
            parts.append(blk.reshape(-1))
        xsp = np.concatenate(parts)
        in_maps.append({"xs": xsp})
    return in_maps


_last_result = None  # test harness can read exec_time_ns / trace from here


def kernel(x, edge_index, edge_attr, batch_size, W, b):
    from concourse import bass_utils

    global _last_result

    x16 = np.asarray(x, dtype=np.float16)
    batch = np.asarray(batch_size).astype(np.int64)
    W = np.asarray(W, dtype=np.float32)
    b = np.asarray(b, dtype=np.float32)

    if batch.size > 1 and np.any(np.diff(batch) < 0):
        # contiguous-shard logic needs sorted ids; reordering nodes does not
        # change per-graph sums
        order = np.argsort(batch, kind="stable")
        batch = batch[order]
        x16 = x16[order]

    counts = np.bincount(batch, minlength=G)
    inv_counts = (1.0 / np.maximum(counts, 1)).astype(np.float32)
    bounds = np.searchsorted(batch, np.arange(0, G + 1, GPC))
    max_rows = int(np.diff(bounds).max())
    ntiles = max(-(-max_rows // P), 1)

    nc = _get_compiled(ntiles)
    in_maps = _prep_in_maps(x16, batch, W, b, ntiles, bounds, inv_counts)

    res = bass_utils.run_bass_kernel_spmd(
        nc, in_maps, core_ids=list(range(NCORES))
    )
    _last_result = res

    # each core returns out.T [7, 128] for its graphs; assemble [1024, 7]
    out = np.concatenate(
        [np.asarray(res.results[k]["out"]).T for k in range(NCORES)], axis=0
    )
    return np.ascontiguousarray(out.astype(np.float32))
